# revision 9
# baseline (speedup 1.0000x reference)
"""Trainium2 Bass kernel for nn_DSNet (dense_cnn) — fp8 DoubleRow version.

Math (see reference): out = sigmoid(relu(relu(conv1(seq_splits)) @ W2 + b2) @ W3 + b3)
where seq = [conv1d(refer, w_seq) ; x^T] split into 32768 (2,512) splits.

Mapping (per core, 4096 splits = 2 super-pairs of (1024 ref + 1024 x) splits):
- ref half: linear_seq+concat+conv1 fold into one stride-2 4-tap conv with
  host-precomputed Weff[d, ch, tau]; computed as DoubleRow fp8 matmuls that
  contract (ch x 2 shifts) x (2 taus in the k-tile dim) = 256 per pass.
- x half: conv1 along the 512-dim becomes 4 banded DoubleRow matmuls with
  M=128 windows + tiny edge-patch matmuls; the 2 k-tiles carry the even/odd
  time rows (host deinterleaves x columns).
- mm2 contracts 512 via DoubleRow over d-block pairs (s1 stored fp8 x16 with
  block pairs side by side in the free dim); mm3 runs bf16 with w3/256 packed
  bit-wise into the f32 bias bundle; sigmoid output fp16.

Scales (powers of 2, exact): refer/x x1 (e4m3), Weff/w1/w2 x16, s1 fp8 x16,
h bf16 x256, w3' = w3/256, biases pre-scaled on host. Measured max rel err
~6.8e-3 vs the f32 reference.

All heavy matmuls are fp8e4 DoubleRow (0.5 cyc/row): ~2.1us ref conv +
~3.4us x conv + ~1.7us mm2 + ~1.7us mm3 of PE per core. Elementwise
(relu/hsb/sigmoid) is spread across Pool/DVE/ACT. DMA ~3.4MB/core fp8.
"""
import sys

import numpy as np

sys.path.insert(0, "/opt/trn_rl_repo")

D_IN, D_SEQ, D_H, D_OUT = 64, 512, 128, 64
T_REF = 32768
N_CORES = 8
NCH = 1024  # splits per chunk (super-pair = ref chunk + x chunk)

_CACHE = {}


def _build_nc():
    import concourse.bacc as bacc
    import concourse.bass as bass
    import concourse.mybir as mybir
    import concourse.tile as tile

    f32 = mybir.dt.float32
    bf16 = mybir.dt.bfloat16
    f16 = mybir.dt.float16
    f8 = mybir.dt.float8e4
    AF = mybir.ActivationFunctionType
    ALU = mybir.AluOpType
    DR = mybir.MatmulPerfMode.DoubleRow

    nc = bacc.Bacc("TRN2", target_bir_lowering=False, debug=False, num_devices=N_CORES)

    refer2_d = nc.dram_tensor("refer2", [128, 2, 2048], f8, kind="ExternalInput").ap()
    xwc_d = nc.dram_tensor("xwc", [2, 128, 4, 2, NCH], f8, kind="ExternalInput").ap()
    xrest_d = nc.dram_tensor("xrest", [2, 2, 2048], f8, kind="ExternalInput").ap()
    w8_d = nc.dram_tensor("w8", [128, 16, 128], f8, kind="ExternalInput").ap()
    wf32_d = nc.dram_tensor("wf32", [128, 40], f32, kind="ExternalInput").ap()
    res_d = nc.dram_tensor("res", [128, 2048], f16, kind="ExternalOutput").ap()

    with tile.TileContext(nc) as tc:
        with (
            tc.tile_pool(name="wp", bufs=1) as wp,
            tc.tile_pool(name="dp", bufs=2) as dp,
            tc.tile_pool(name="s1p", bufs=8) as s1p,
            tc.tile_pool(name="hp", bufs=2) as hp,
            tc.tile_pool(name="pp", bufs=4, space=bass.MemorySpace.PSUM) as pp,
        ):
            # --- PE / ACT warmup: dummy matmuls ramp the PE pstate and the
            # activations preload the Relu/Sigmoid tables while DMAs fly.
            warm = wp.tile([1, 512], f8)
            nc.gpsimd.memset(warm[:], 0.0)
            # first-needed weights via SWDGE (parallel to the HWDGE queue)
            w8 = wp.tile([128, 16, 128], f8)
            nc.gpsimd.dma_start(w8[:, 0:8, :], w8_d[:, 0:8, :])
            wf32 = wp.tile([128, 40], f32)
            nc.gpsimd.dma_start(wf32[:], wf32_d[:])
            xrest = wp.tile([2, 2, 2048], f8)
            nc.gpsimd.dma_start(xrest[:], xrest_d[:])

            wact = wp.tile([1, 16], f32)
            nc.scalar.activation(wact[:], warm[0:1, 0:16], AF.Relu)
            nc.scalar.activation(wact[:], warm[0:1, 0:16], AF.Sigmoid)
            psw = pp.tile([1, NCH], f32, tag="ps", name="psw")
            for _ in range(5):
                nc.tensor.matmul(
                    psw[0:1, 0:512], warm[0:1, 0:1], warm[0:1, :], start=True,
                    stop=True,
                )

            refer2 = wp.tile([128, 2, 2048], f8)
            nc.sync.dma_start(refer2[:, :, 0:512], refer2_d[:, :, 0:512])
            nc.sync.dma_start(w8[:, 8:12, :], w8_d[:, 8:12, :])

            xt_tiles = {}

            def load_xt(b):
                xt = dp.tile([128, 4, 2, NCH], f8, tag="xt", name=f"xt_{b}")
                nc.sync.dma_start(xt[:], xwc_d[b])
                xt_tiles[b] = xt

            load_xt(0)
            nc.sync.dma_start(refer2[:, :, 512:2048], refer2_d[:, :, 512:2048])
            nc.sync.dma_start(w8[:, 12:16, :], w8_d[:, 12:16, :])
            load_xt(1)

            res = wp.tile([128, 2048], f16)
            wb = wf32[:, 8:40].bitcast(bf16)  # [128, 64] w3/256

            # psum->sbuf relu ops: only ACT and DVE can read PSUM
            def relu_op(eng, out, psum, bias):
                if eng == "a":
                    nc.scalar.activation(out, psum, AF.Relu, bias=bias)
                else:
                    nc.vector.tensor_scalar(out, psum, bias, 0.0, ALU.add, ALU.max)

            s1r_tiles, s1x_tiles, hcat_tiles = {}, {}, {}

            def conv_ref(s):
                s1r = [
                    s1p.tile([128, 2, NCH], f8, tag="s1", name=f"s1r{g}_{s}")
                    for g in range(2)
                ]
                for q in range(4):
                    ps = pp.tile([128, NCH], f32, tag="ps", name=f"psr_{s}_{q}")
                    for nh in range(4):
                        nc.tensor.matmul(
                            ps[:, 256 * nh:256 * (nh + 1)],
                            w8[:, 2 * q:2 * q + 2, :],
                            refer2[:, :, NCH * s + 256 * nh:NCH * s + 256 * (nh + 1)],
                            start=True, stop=True, perf_mode=DR,
                        )
                    relu_op("av"[q % 2], s1r[q // 2][:, q % 2, :], ps[:],
                            wf32[:, q:q + 1])
                s1r_tiles[s] = s1r

            def conv_x_win(s, j, eng):
                if j == 0:
                    s1x_tiles[s] = [
                        s1p.tile([128, 2, NCH], f8, tag="s1", name=f"s1x{g}_{s}")
                        for g in range(2)
                    ]
                xt = xt_tiles[s]
                ps = pp.tile([128, NCH], f32, tag="ps", name=f"psx_{s}_{j}")
                for nh in range(4):
                    o = 256 * nh
                    nc.tensor.matmul(
                        ps[:, o:o + 256],
                        w8[:, 8:10, :],
                        xt[:, j, :, o:o + 256],
                        start=True, stop=False, perf_mode=DR,
                    )
                    if j < 3:
                        prhs = xt[0:2, j + 1, :, o:o + 256]
                    else:
                        prhs = xrest[0:2, :, NCH * s + o:NCH * s + o + 256]
                    nc.tensor.matmul(
                        ps[:, o:o + 256],
                        w8[0:2, 10:12, :],
                        prhs,
                        start=False, stop=True, perf_mode=DR,
                    )
                relu_op(eng, s1x_tiles[s][j // 2][:, j % 2, :], ps[:],
                        wf32[:, 6:7])

            def mm2_half(ph, s1t, c0, cols):
                """mm2 psum cols [c0, c0+cols) from s1 tiles."""
                for o in range(c0, c0 + cols, 256):
                    for g in range(2):
                        nc.tensor.matmul(
                            ph[:, o:o + 256],
                            w8[:, 12 + 2 * g:14 + 2 * g, :],
                            s1t[g][:, :, o:o + 256],
                            start=(g == 0), stop=(g == 1), perf_mode=DR,
                        )

            def tail_mm2(s):
                hcat = hp.tile([128, 2, NCH], bf16, tag="h", name=f"hcat_{s}")
                hcat_tiles[s] = hcat
                phr = pp.tile([128, NCH], f32, tag="ps", name=f"pshr_{s}")
                mm2_half(phr, s1r_tiles[s], 0, NCH)
                relu_op("a", hcat[:, 0, :], phr[:], wf32[:, 5:6])
                phx = pp.tile([128, NCH], f32, tag="ps", name=f"pshx_{s}")
                mm2_half(phx, s1x_tiles[s], 0, NCH)
                relu_op("v", hcat[:, 1, :], phx[:], wf32[:, 5:6])

            def tail_mm3(s):
                hcat = hcat_tiles[s]
                pso = pp.tile([128, NCH], f32, tag="ps", name=f"pso_{s}")
                for half in range(2):
                    for n2 in range(2):
                        nc.tensor.matmul(
                            pso[64 * half:64 * half + 64, 512 * n2:512 * (n2 + 1)],
                            wb,
                            hcat[:, half, 512 * n2:512 * (n2 + 1)],
                            start=True, stop=True,
                            tile_position=(0, 64 * half),
                        )
                nc.scalar.activation(
                    res[:, NCH * s:NCH * (s + 1)], pso[:], AF.Sigmoid,
                    bias=wf32[:, 4:5],
                )
                nc.sync.dma_start(
                    res_d[:, NCH * s:NCH * (s + 1)], res[:, NCH * s:NCH * (s + 1)]
                )

            def tail_split(s):
                """Last super-pair: pipelined 512-col half-tails to shrink the
                end-of-program latency chain."""
                hcat = hp.tile([128, 2, NCH], bf16, tag="h", name=f"hcat_{s}")
                for n2 in range(2):
                    c0 = 512 * n2
                    phr = pp.tile([128, NCH], f32, tag="ps", name=f"pshr_{s}_{n2}")
                    mm2_half(phr, s1r_tiles[s], c0, 512)
                    relu_op("a", hcat[:, 0, c0:c0 + 512], phr[:, c0:c0 + 512],
                            wf32[:, 5:6])
                    phx = pp.tile([128, NCH], f32, tag="ps", name=f"pshx_{s}_{n2}")
                    mm2_half(phx, s1x_tiles[s], c0, 512)
                    relu_op("v", hcat[:, 1, c0:c0 + 512], phx[:, c0:c0 + 512],
                            wf32[:, 5:6])
                    pso = pp.tile([128, NCH], f32, tag="ps", name=f"pso_{s}_{n2}")
                    for half in range(2):
                        nc.tensor.matmul(
                            pso[64 * half:64 * half + 64, c0:c0 + 512],
                            wb,
                            hcat[:, half, c0:c0 + 512],
                            start=True, stop=True,
                            tile_position=(0, 64 * half),
                        )
                    nc.scalar.activation(
                        res[:, NCH * s + c0:NCH * s + c0 + 512],
                        pso[:, c0:c0 + 512], AF.Sigmoid, bias=wf32[:, 4:5],
                    )
                    nc.sync.dma_start(
                        res_d[:, NCH * s + c0:NCH * s + c0 + 512],
                        res[:, NCH * s + c0:NCH * s + c0 + 512],
                    )

            conv_ref(0)
            for j in range(4):
                conv_x_win(0, j, "av"[j % 2])
            conv_ref(1)
            tail_mm2(0)
            conv_x_win(1, 0, "a")
            conv_x_win(1, 1, "v")
            tail_mm3(0)
            conv_x_win(1, 2, "a")
            conv_x_win(1, 3, "v")
            tail_split(1)

    nc.compile()
    return nc


def _host_prep_weights(w_seq, b_seq, w1, b1, w2, b2, w3, b3):
    import ml_dtypes

    E4 = ml_dtypes.float8_e4m3
    BF = ml_dtypes.bfloat16

    w_seq64 = np.asarray(w_seq, np.float64)
    b_seq64 = np.asarray(b_seq, np.float64)
    w164 = np.asarray(w1, np.float64)

    Weff = np.zeros((D_SEQ, D_IN, 4))
    beff = np.full(D_SEQ, float(np.asarray(b1).reshape(-1)[0]))
    for cc in (0, 1):
        for k in range(3):
            dlo, dhi = max(0, 1 - k), min(D_SEQ, D_SEQ + 1 - k)
            for kk in range(3):
                tau = cc + kk
                Weff[dlo:dhi, :, tau] += (
                    w164[0, cc, k] * w_seq64[dlo + k - 1:dhi + k - 1, :, kk]
                )
    for k in range(3):
        dlo, dhi = max(0, 1 - k), min(D_SEQ, D_SEQ + 1 - k)
        beff[dlo:dhi] += (w164[0, 0, k] + w164[0, 1, k]) * b_seq64[dlo + k - 1:dhi + k - 1]

    w8 = np.zeros((128, 16, 128), np.float64)
    # wefft groups 2q+i: w8[64*bb+ch, 2q+i, m] = 16*Weff[128q+m, ch, 2bb+i]
    for q in range(4):
        for bb in range(2):
            for i in range(2):
                w8[64 * bb:64 * bb + 64, 2 * q + i, :] = (
                    16.0 * Weff[128 * q:128 * (q + 1), :, 2 * bb + i].T
                )
    # band groups 8+i: w8[p, 8+i, m] = 16*w1[0,i,p-m], p-m in {0,1,2}
    for i in range(2):
        for m in range(128):
            for k in range(3):
                if m + k < 128:
                    w8[m + k, 8 + i, m] = 16.0 * w164[0, i, k]
        # patch groups 10+i
        w8[0, 10 + i, 126] = 16.0 * w164[0, i, 2]
        w8[0, 10 + i, 127] = 16.0 * w164[0, i, 1]
        w8[1, 10 + i, 127] = 16.0 * w164[0, i, 2]
    # w2 groups 12+2g+i: w8[p, 12+2g+i, e] = 16*w2[e, 128*(2g+i)+p]
    w2m = np.asarray(w2, np.float64)[:, :, 1]  # (128, 512)
    for g in range(2):
        for i in range(2):
            blk = 2 * g + i
            w8[:, 12 + 2 * g + i, :] = 16.0 * w2m[:, 128 * blk:128 * (blk + 1)].T

    wf32 = np.zeros((128, 40), np.float32)
    for q in range(4):
        wf32[:, q] = 16.0 * beff[128 * q:128 * (q + 1)]
    b3a = np.asarray(b3, np.float64)
    wf32[:, 4] = np.tile(b3a, 2).astype(np.float32)
    wf32[:, 5] = 256.0 * np.asarray(b2, np.float64)
    wf32[:, 6] = 16.0 * float(np.asarray(b1).reshape(-1)[0])
    # w3' = w3/256 as bf16, bit-packed into f32 columns 8..40
    w3p = np.ascontiguousarray((np.asarray(w3, np.float64)[:, :, 1].T / 256.0)).astype(BF)
    wf32[:, 8:40] = w3p.view(np.float32)

    return np.ascontiguousarray(w8.astype(E4)), np.ascontiguousarray(wf32)


def _host_prep_data(refer, x):
    """Global fp8 conversion + padded transposes shared by all cores."""
    import ml_dtypes

    E4 = ml_dtypes.float8_e4m3
    refer8p = np.zeros((D_IN, T_REF + 4), E4)
    refer8p[:, 1:T_REF + 1] = np.asarray(refer[0], np.float32).astype(E4)
    x8 = np.asarray(x[0], np.float32).astype(E4)  # (T, 512)
    xTpad = np.zeros((513, T_REF), E4)
    xTpad[1:513] = x8.T
    return refer8p, xTpad


def _host_prep_core(c, refer8p, xTpad):
    import ml_dtypes

    E4 = ml_dtypes.float8_e4m3
    # refer2[p, i, cn] = refer[ch, 4096c + 2cn + i - 1 + 2*(p>=64)]
    # refer8p col t -> index t+1
    refer2 = np.zeros((128, 2, 2048), E4)
    base = 4096 * c
    for i in range(2):
        refer2[0:64, i, :] = refer8p[:, base + i:base + i + 4096:2]
        refer2[64:128, i, :] = refer8p[:, base + i + 2:base + i + 4096 + 2:2]
    # xwc[b, p, j, i, n] = xTpad[128j + p, 4096c + 2048b + 2n + i]
    xwc = np.zeros((2, 128, 4, 2, NCH), E4)
    for b in range(2):
        t0 = 4096 * c + 2048 * b
        for j in range(4):
            blk = xTpad[128 * j:128 * j + 128, t0:t0 + 2048]
            xwc[b, :, j, 0, :] = blk[:, 0::2]
            xwc[b, :, j, 1, :] = blk[:, 1::2]
    xrest = np.zeros((2, 2, 2048), E4)
    xrest[0, 0, :] = xTpad[512, 4096 * c:4096 * (c + 1):2]
    xrest[0, 1, :] = xTpad[512, 4096 * c + 1:4096 * (c + 1):2]
    return refer2, np.ascontiguousarray(xwc), xrest


def kernel(refer, x, w_seq, b_seq, w1, b1, w2, b2, w3, b3):
    from concourse.bass_utils import run_bass_kernel_spmd

    if "nc" not in _CACHE:
        _CACHE["nc"] = _build_nc()
    nc = _CACHE["nc"]

    w8, wf32 = _host_prep_weights(w_seq, b_seq, w1, b1, w2, b2, w3, b3)
    refer8p, xTpad = _host_prep_data(refer, x)
    in_maps = []
    for c in range(N_CORES):
        refer2, xwc, xrest = _host_prep_core(c, refer8p, xTpad)
        in_maps.append(
            dict(refer2=refer2, xwc=xwc, xrest=xrest, w8=w8, wf32=wf32)
        )

    res = run_bass_kernel_spmd(nc, in_maps, core_ids=list(range(N_CORES)))

    final = np.zeros((32768, D_OUT, 1), np.float32)
    for c in range(N_CORES):
        r = np.asarray(res.results[c]["res"], np.float32)  # (128, 2048)
        final[2048 * c:2048 * (c + 1), :, 0] = r[0:64, :].T
        final[16384 + 2048 * c:16384 + 2048 * (c + 1), :, 0] = r[64:128, :].T
    return final


# revision 10
# speedup vs baseline: 1.0679x; 1.0679x over previous
"""Trainium2 Bass kernel for nn_DSNet (dense_cnn) — fp8 DoubleRow version.

Math (see reference): out = sigmoid(relu(relu(conv1(seq_splits)) @ W2 + b2) @ W3 + b3)
where seq = [conv1d(refer, w_seq) ; x^T] split into 32768 (2,512) splits.

Mapping (per core, 4096 splits = 2 super-pairs of (1024 ref + 1024 x) splits):
- ref half: linear_seq+concat+conv1 fold into one stride-2 4-tap conv with
  host-precomputed Weff[d, ch, tau]; computed as DoubleRow fp8 matmuls that
  contract (ch x 2 shifts) x (2 taus in the k-tile dim) = 256 per pass.
- x half: conv1 along the 512-dim becomes 4 banded DoubleRow matmuls with
  M=128 windows + tiny edge-patch matmuls; the 2 k-tiles carry the even/odd
  time rows (host deinterleaves x columns).
- mm2 contracts 512 via DoubleRow over d-block pairs (s1 stored fp8 x16 with
  block pairs side by side in the free dim); mm3 runs bf16 with w3/256 packed
  bit-wise into the f32 bias bundle; sigmoid output fp16.

Scales (powers of 2, exact): refer/x x1 (e4m3), Weff/w1/w2 x16, s1 fp8 x16,
h bf16 x256, w3' = w3/256, biases pre-scaled on host. Measured max rel err
~6.8e-3 vs the f32 reference.

All heavy matmuls are fp8e4 DoubleRow (0.5 cyc/row): ~2.1us ref conv +
~3.4us x conv + ~1.7us mm2 + ~1.7us mm3 of PE per core. Elementwise
(relu/hsb/sigmoid) is spread across Pool/DVE/ACT. DMA ~3.4MB/core fp8.
"""
import sys

import numpy as np

sys.path.insert(0, "/opt/trn_rl_repo")

D_IN, D_SEQ, D_H, D_OUT = 64, 512, 128, 64
T_REF = 32768
N_CORES = 8
NCH = 1024  # splits per chunk (super-pair = ref chunk + x chunk)

_CACHE = {}


def _build_nc():
    import concourse.bacc as bacc
    import concourse.bass as bass
    import concourse.mybir as mybir
    import concourse.tile as tile

    f32 = mybir.dt.float32
    bf16 = mybir.dt.bfloat16
    f16 = mybir.dt.float16
    f8 = mybir.dt.float8e4
    AF = mybir.ActivationFunctionType
    ALU = mybir.AluOpType
    DR = mybir.MatmulPerfMode.DoubleRow

    nc = bacc.Bacc("TRN2", target_bir_lowering=False, debug=False, num_devices=N_CORES)

    refer2_d = nc.dram_tensor("refer2", [128, 2, 2048], f8, kind="ExternalInput").ap()
    xwc_d = nc.dram_tensor("xwc", [2, 128, 4, 2, NCH], f8, kind="ExternalInput").ap()
    xrest_d = nc.dram_tensor("xrest", [2, 2, 2048], f8, kind="ExternalInput").ap()
    w8_d = nc.dram_tensor("w8", [128, 16, 128], f8, kind="ExternalInput").ap()
    wf32_d = nc.dram_tensor("wf32", [128, 40], f32, kind="ExternalInput").ap()
    res_d = nc.dram_tensor("res", [128, 2048], f16, kind="ExternalOutput").ap()

    with tile.TileContext(nc) as tc:
        with (
            tc.tile_pool(name="wp", bufs=1) as wp,
            tc.tile_pool(name="dp", bufs=2) as dp,
            tc.tile_pool(name="s1p", bufs=8) as s1p,
            tc.tile_pool(name="hp", bufs=2) as hp,
            tc.tile_pool(name="pp", bufs=4, space=bass.MemorySpace.PSUM) as pp,
        ):
            # --- PE / ACT warmup: dummy matmuls ramp the PE pstate and the
            # activations preload the Relu/Sigmoid tables while DMAs fly.
            warm = wp.tile([1, 512], f8)
            nc.gpsimd.memset(warm[:], 0.0)
            # first-needed weights via SWDGE (parallel to the HWDGE queue)
            w8 = wp.tile([128, 16, 128], f8)
            nc.gpsimd.dma_start(w8[:, 0:8, :], w8_d[:, 0:8, :])
            wf32 = wp.tile([128, 40], f32)
            nc.gpsimd.dma_start(wf32[:], wf32_d[:])
            xrest = wp.tile([2, 2, 2048], f8)
            nc.gpsimd.dma_start(xrest[:], xrest_d[:])

            wact = wp.tile([1, 16], f32)
            nc.scalar.activation(wact[:], warm[0:1, 0:16], AF.Relu)
            nc.scalar.activation(wact[:], warm[0:1, 0:16], AF.Sigmoid)
            psw = pp.tile([1, NCH], f32, tag="ps", name="psw")
            for _ in range(7):
                nc.tensor.matmul(
                    psw[0:1, 0:512], warm[0:1, 0:1], warm[0:1, :], start=True,
                    stop=True,
                )

            refer2 = wp.tile([128, 2, 2048], f8)
            nc.sync.dma_start(refer2[:, :, 0:1024], refer2_d[:, :, 0:1024])
            nc.sync.dma_start(w8[:, 8:12, :], w8_d[:, 8:12, :])

            xt_tiles = {}

            def load_xt(b):
                xt = dp.tile([128, 4, 2, NCH], f8, tag="xt", name=f"xt_{b}")
                nc.sync.dma_start(xt[:], xwc_d[b])
                xt_tiles[b] = xt

            load_xt(0)
            nc.sync.dma_start(refer2[:, :, 1024:2048], refer2_d[:, :, 1024:2048])
            nc.sync.dma_start(w8[:, 12:16, :], w8_d[:, 12:16, :])
            load_xt(1)

            res = wp.tile([128, 2048], f16)
            wb = wf32[:, 8:40].bitcast(bf16)  # [128, 64] w3/256

            # psum->sbuf relu ops: only ACT and DVE can read PSUM
            def relu_op(eng, out, psum, bias):
                if eng == "a":
                    nc.scalar.activation(out, psum, AF.Relu, bias=bias)
                else:
                    nc.vector.tensor_scalar(out, psum, bias, 0.0, ALU.add, ALU.max)

            s1r_tiles, s1x_tiles, hcat_tiles = {}, {}, {}

            def conv_ref(s):
                s1r = [
                    s1p.tile([128, 2, NCH], f8, tag="s1", name=f"s1r{g}_{s}")
                    for g in range(2)
                ]
                for q in range(4):
                    ps = pp.tile([128, NCH], f32, tag="ps", name=f"psr_{s}_{q}")
                    for nh in range(4):
                        nc.tensor.matmul(
                            ps[:, 256 * nh:256 * (nh + 1)],
                            w8[:, 2 * q:2 * q + 2, :],
                            refer2[:, :, NCH * s + 256 * nh:NCH * s + 256 * (nh + 1)],
                            start=True, stop=True, perf_mode=DR,
                        )
                    relu_op("av"[q % 2], s1r[q // 2][:, q % 2, :], ps[:],
                            wf32[:, q:q + 1])
                s1r_tiles[s] = s1r

            def conv_x_win(s, j, eng):
                if j == 0:
                    s1x_tiles[s] = [
                        s1p.tile([128, 2, NCH], f8, tag="s1", name=f"s1x{g}_{s}")
                        for g in range(2)
                    ]
                xt = xt_tiles[s]
                ps = pp.tile([128, NCH], f32, tag="ps", name=f"psx_{s}_{j}")
                for nh in range(4):
                    o = 256 * nh
                    nc.tensor.matmul(
                        ps[:, o:o + 256],
                        w8[:, 8:10, :],
                        xt[:, j, :, o:o + 256],
                        start=True, stop=False, perf_mode=DR,
                    )
                    if j < 3:
                        prhs = xt[0:2, j + 1, :, o:o + 256]
                    else:
                        prhs = xrest[0:2, :, NCH * s + o:NCH * s + o + 256]
                    nc.tensor.matmul(
                        ps[:, o:o + 256],
                        w8[0:2, 10:12, :],
                        prhs,
                        start=False, stop=True, perf_mode=DR,
                    )
                relu_op(eng, s1x_tiles[s][j // 2][:, j % 2, :], ps[:],
                        wf32[:, 6:7])

            def mm2_half(ph, s1t, c0, cols):
                """mm2 psum cols [c0, c0+cols) from s1 tiles."""
                for o in range(c0, c0 + cols, 256):
                    for g in range(2):
                        nc.tensor.matmul(
                            ph[:, o:o + 256],
                            w8[:, 12 + 2 * g:14 + 2 * g, :],
                            s1t[g][:, :, o:o + 256],
                            start=(g == 0), stop=(g == 1), perf_mode=DR,
                        )

            def tail_mm2(s):
                hcat = hp.tile([128, 2, NCH], bf16, tag="h", name=f"hcat_{s}")
                hcat_tiles[s] = hcat
                phr = pp.tile([128, NCH], f32, tag="ps", name=f"pshr_{s}")
                mm2_half(phr, s1r_tiles[s], 0, NCH)
                relu_op("a", hcat[:, 0, :], phr[:], wf32[:, 5:6])
                phx = pp.tile([128, NCH], f32, tag="ps", name=f"pshx_{s}")
                mm2_half(phx, s1x_tiles[s], 0, NCH)
                relu_op("v", hcat[:, 1, :], phx[:], wf32[:, 5:6])

            def tail_mm3(s):
                hcat = hcat_tiles[s]
                pso = pp.tile([128, NCH], f32, tag="ps", name=f"pso_{s}")
                for half in range(2):
                    for n2 in range(2):
                        nc.tensor.matmul(
                            pso[64 * half:64 * half + 64, 512 * n2:512 * (n2 + 1)],
                            wb,
                            hcat[:, half, 512 * n2:512 * (n2 + 1)],
                            start=True, stop=True,
                            tile_position=(0, 64 * half),
                        )
                nc.scalar.activation(
                    res[:, NCH * s:NCH * (s + 1)], pso[:], AF.Sigmoid,
                    bias=wf32[:, 4:5],
                )
                nc.sync.dma_start(
                    res_d[:, NCH * s:NCH * (s + 1)], res[:, NCH * s:NCH * (s + 1)]
                )

            def tail_split(s):
                """Last super-pair: pipelined 512-col half-tails to shrink the
                end-of-program latency chain."""
                hcat = hp.tile([128, 2, NCH], bf16, tag="h", name=f"hcat_{s}")
                for n2 in range(2):
                    c0 = 512 * n2
                    phr = pp.tile([128, NCH], f32, tag="ps", name=f"pshr_{s}_{n2}")
                    mm2_half(phr, s1r_tiles[s], c0, 512)
                    relu_op("a", hcat[:, 0, c0:c0 + 512], phr[:, c0:c0 + 512],
                            wf32[:, 5:6])
                    phx = pp.tile([128, NCH], f32, tag="ps", name=f"pshx_{s}_{n2}")
                    mm2_half(phx, s1x_tiles[s], c0, 512)
                    relu_op("v", hcat[:, 1, c0:c0 + 512], phx[:, c0:c0 + 512],
                            wf32[:, 5:6])
                    pso = pp.tile([128, NCH], f32, tag="ps", name=f"pso_{s}_{n2}")
                    for half in range(2):
                        nc.tensor.matmul(
                            pso[64 * half:64 * half + 64, c0:c0 + 512],
                            wb,
                            hcat[:, half, c0:c0 + 512],
                            start=True, stop=True,
                            tile_position=(0, 64 * half),
                        )
                    nc.scalar.activation(
                        res[:, NCH * s + c0:NCH * s + c0 + 512],
                        pso[:, c0:c0 + 512], AF.Sigmoid, bias=wf32[:, 4:5],
                    )
                    nc.sync.dma_start(
                        res_d[:, NCH * s + c0:NCH * s + c0 + 512],
                        res[:, NCH * s + c0:NCH * s + c0 + 512],
                    )

            conv_ref(0)
            for j in range(4):
                conv_x_win(0, j, "av"[j % 2])
            conv_ref(1)
            tail_mm2(0)
            conv_x_win(1, 0, "a")
            conv_x_win(1, 1, "v")
            tail_mm3(0)
            conv_x_win(1, 2, "a")
            conv_x_win(1, 3, "v")
            tail_split(1)

    nc.compile()
    return nc


def _host_prep_weights(w_seq, b_seq, w1, b1, w2, b2, w3, b3):
    import ml_dtypes

    E4 = ml_dtypes.float8_e4m3
    BF = ml_dtypes.bfloat16

    w_seq64 = np.asarray(w_seq, np.float64)
    b_seq64 = np.asarray(b_seq, np.float64)
    w164 = np.asarray(w1, np.float64)

    Weff = np.zeros((D_SEQ, D_IN, 4))
    beff = np.full(D_SEQ, float(np.asarray(b1).reshape(-1)[0]))
    for cc in (0, 1):
        for k in range(3):
            dlo, dhi = max(0, 1 - k), min(D_SEQ, D_SEQ + 1 - k)
            for kk in range(3):
                tau = cc + kk
                Weff[dlo:dhi, :, tau] += (
                    w164[0, cc, k] * w_seq64[dlo + k - 1:dhi + k - 1, :, kk]
                )
    for k in range(3):
        dlo, dhi = max(0, 1 - k), min(D_SEQ, D_SEQ + 1 - k)
        beff[dlo:dhi] += (w164[0, 0, k] + w164[0, 1, k]) * b_seq64[dlo + k - 1:dhi + k - 1]

    w8 = np.zeros((128, 16, 128), np.float64)
    # wefft groups 2q+i: w8[64*bb+ch, 2q+i, m] = 16*Weff[128q+m, ch, 2bb+i]
    for q in range(4):
        for bb in range(2):
            for i in range(2):
                w8[64 * bb:64 * bb + 64, 2 * q + i, :] = (
                    16.0 * Weff[128 * q:128 * (q + 1), :, 2 * bb + i].T
                )
    # band groups 8+i: w8[p, 8+i, m] = 16*w1[0,i,p-m], p-m in {0,1,2}
    for i in range(2):
        for m in range(128):
            for k in range(3):
                if m + k < 128:
                    w8[m + k, 8 + i, m] = 16.0 * w164[0, i, k]
        # patch groups 10+i
        w8[0, 10 + i, 126] = 16.0 * w164[0, i, 2]
        w8[0, 10 + i, 127] = 16.0 * w164[0, i, 1]
        w8[1, 10 + i, 127] = 16.0 * w164[0, i, 2]
    # w2 groups 12+2g+i: w8[p, 12+2g+i, e] = 16*w2[e, 128*(2g+i)+p]
    w2m = np.asarray(w2, np.float64)[:, :, 1]  # (128, 512)
    for g in range(2):
        for i in range(2):
            blk = 2 * g + i
            w8[:, 12 + 2 * g + i, :] = 16.0 * w2m[:, 128 * blk:128 * (blk + 1)].T

    wf32 = np.zeros((128, 40), np.float32)
    for q in range(4):
        wf32[:, q] = 16.0 * beff[128 * q:128 * (q + 1)]
    b3a = np.asarray(b3, np.float64)
    wf32[:, 4] = np.tile(b3a, 2).astype(np.float32)
    wf32[:, 5] = 256.0 * np.asarray(b2, np.float64)
    wf32[:, 6] = 16.0 * float(np.asarray(b1).reshape(-1)[0])
    # w3' = w3/256 as bf16, bit-packed into f32 columns 8..40
    w3p = np.ascontiguousarray((np.asarray(w3, np.float64)[:, :, 1].T / 256.0)).astype(BF)
    wf32[:, 8:40] = w3p.view(np.float32)

    return np.ascontiguousarray(w8.astype(E4)), np.ascontiguousarray(wf32)


def _host_prep_data(refer, x):
    """Global fp8 conversion + padded transposes shared by all cores."""
    import ml_dtypes

    E4 = ml_dtypes.float8_e4m3
    refer8p = np.zeros((D_IN, T_REF + 4), E4)
    refer8p[:, 1:T_REF + 1] = np.asarray(refer[0], np.float32).astype(E4)
    x8 = np.asarray(x[0], np.float32).astype(E4)  # (T, 512)
    xTpad = np.zeros((513, T_REF), E4)
    xTpad[1:513] = x8.T
    return refer8p, xTpad


def _host_prep_core(c, refer8p, xTpad):
    import ml_dtypes

    E4 = ml_dtypes.float8_e4m3
    # refer2[p, i, cn] = refer[ch, 4096c + 2cn + i - 1 + 2*(p>=64)]
    # refer8p col t -> index t+1
    refer2 = np.zeros((128, 2, 2048), E4)
    base = 4096 * c
    for i in range(2):
        refer2[0:64, i, :] = refer8p[:, base + i:base + i + 4096:2]
        refer2[64:128, i, :] = refer8p[:, base + i + 2:base + i + 4096 + 2:2]
    # xwc[b, p, j, i, n] = xTpad[128j + p, 4096c + 2048b + 2n + i]
    xwc = np.zeros((2, 128, 4, 2, NCH), E4)
    for b in range(2):
        t0 = 4096 * c + 2048 * b
        for j in range(4):
            blk = xTpad[128 * j:128 * j + 128, t0:t0 + 2048]
            xwc[b, :, j, 0, :] = blk[:, 0::2]
            xwc[b, :, j, 1, :] = blk[:, 1::2]
    xrest = np.zeros((2, 2, 2048), E4)
    xrest[0, 0, :] = xTpad[512, 4096 * c:4096 * (c + 1):2]
    xrest[0, 1, :] = xTpad[512, 4096 * c + 1:4096 * (c + 1):2]
    return refer2, np.ascontiguousarray(xwc), xrest


def kernel(refer, x, w_seq, b_seq, w1, b1, w2, b2, w3, b3):
    from concourse.bass_utils import run_bass_kernel_spmd

    if "nc" not in _CACHE:
        _CACHE["nc"] = _build_nc()
    nc = _CACHE["nc"]

    w8, wf32 = _host_prep_weights(w_seq, b_seq, w1, b1, w2, b2, w3, b3)
    refer8p, xTpad = _host_prep_data(refer, x)
    in_maps = []
    for c in range(N_CORES):
        refer2, xwc, xrest = _host_prep_core(c, refer8p, xTpad)
        in_maps.append(
            dict(refer2=refer2, xwc=xwc, xrest=xrest, w8=w8, wf32=wf32)
        )

    res = run_bass_kernel_spmd(nc, in_maps, core_ids=list(range(N_CORES)))

    final = np.zeros((32768, D_OUT, 1), np.float32)
    for c in range(N_CORES):
        r = np.asarray(res.results[c]["res"], np.float32)  # (128, 2048)
        final[2048 * c:2048 * (c + 1), :, 0] = r[0:64, :].T
        final[16384 + 2048 * c:16384 + 2048 * (c + 1), :, 0] = r[64:128, :].T
    return final


# revision 11
# speedup vs baseline: 1.1444x; 1.0717x over previous
"""Trainium2 Bass kernel for nn_DSNet (dense_cnn) — fp8 DoubleRow version.

Math (see reference): out = sigmoid(relu(relu(conv1(seq_splits)) @ W2 + b2) @ W3 + b3)
where seq = [conv1d(refer, w_seq) ; x^T] split into 32768 (2,512) splits.

Mapping (per core, 4096 splits = 2 super-pairs of (1024 ref + 1024 x) splits):
- ref half: linear_seq+concat+conv1 fold into one stride-2 4-tap conv with
  host-precomputed Weff[d, ch, tau]; computed as DoubleRow fp8 matmuls that
  contract (ch x 2 shifts) x (2 taus in the k-tile dim) = 256 per pass.
- x half: conv1 along the 512-dim becomes 4 banded DoubleRow matmuls with
  M=128 windows + tiny edge-patch matmuls; the 2 k-tiles carry the even/odd
  time rows (host deinterleaves x columns).
- mm2 contracts 512 via DoubleRow over d-block pairs (s1 stored fp8 x16 with
  block pairs side by side in the free dim); mm3 runs bf16 with w3/256 packed
  bit-wise into the f32 bias bundle; sigmoid output fp16.

Scales (powers of 2, exact): refer/x x1 (e4m3), Weff/w1/w2 x16, s1 fp8 x16,
h bf16 x256, w3' = w3/256, biases pre-scaled on host. Measured max rel err
~6.8e-3 vs the f32 reference.

All heavy matmuls are fp8e4 DoubleRow (0.5 cyc/row): ~2.1us ref conv +
~3.4us x conv + ~1.7us mm2 + ~1.7us mm3 of PE per core. Elementwise
(relu/hsb/sigmoid) is spread across Pool/DVE/ACT. DMA ~3.4MB/core fp8.
"""
import sys

import numpy as np

sys.path.insert(0, "/opt/trn_rl_repo")

D_IN, D_SEQ, D_H, D_OUT = 64, 512, 128, 64
T_REF = 32768
N_CORES = 8
NCH = 1024  # splits per chunk (super-pair = ref chunk + x chunk)

_CACHE = {}


def _build_nc():
    import concourse.bacc as bacc
    import concourse.bass as bass
    import concourse.mybir as mybir
    import concourse.tile as tile

    f32 = mybir.dt.float32
    bf16 = mybir.dt.bfloat16
    f16 = mybir.dt.float16
    f8 = mybir.dt.float8e4
    AF = mybir.ActivationFunctionType
    ALU = mybir.AluOpType
    DR = mybir.MatmulPerfMode.DoubleRow

    nc = bacc.Bacc("TRN2", target_bir_lowering=False, debug=False, num_devices=N_CORES)

    refer2_d = nc.dram_tensor("refer2", [128, 2, 2048], f8, kind="ExternalInput").ap()
    xwc_d = nc.dram_tensor("xwc", [2, 128, 4, 2, NCH], f8, kind="ExternalInput").ap()
    xrest_d = nc.dram_tensor("xrest", [2, 2, 2048], f8, kind="ExternalInput").ap()
    w8_d = nc.dram_tensor("w8", [128, 16, 128], f8, kind="ExternalInput").ap()
    wf32_d = nc.dram_tensor("wf32", [128, 40], f32, kind="ExternalInput").ap()
    res_d = nc.dram_tensor("res", [128, 2048], f16, kind="ExternalOutput").ap()

    with tile.TileContext(nc) as tc:
        with (
            tc.tile_pool(name="wp", bufs=1) as wp,
            tc.tile_pool(name="dp", bufs=2) as dp,
            tc.tile_pool(name="s1p", bufs=8) as s1p,
            tc.tile_pool(name="hp", bufs=2) as hp,
            tc.tile_pool(name="pp", bufs=4, space=bass.MemorySpace.PSUM) as pp,
        ):
            # --- PE / ACT warmup: dummy matmuls ramp the PE pstate and the
            # activations preload the Relu/Sigmoid tables while DMAs fly.
            warm = wp.tile([1, 512], f8)
            nc.gpsimd.memset(warm[:], 0.0)
            xrest = wp.tile([2, 2, 2048], f8)
            nc.gpsimd.dma_start(xrest[:], xrest_d[:])

            # critical-path loads on SP/HWDGE in first-needed order
            w8 = wp.tile([128, 16, 128], f8)
            refer2 = wp.tile([128, 2, 2048], f8)
            wf32 = wp.tile([128, 40], f32)
            nc.sync.dma_start(w8[:, 0:8, :], w8_d[:, 0:8, :])
            nc.sync.dma_start(refer2[:, :, 0:1024], refer2_d[:, :, 0:1024])
            nc.sync.dma_start(wf32[:], wf32_d[:])

            wact = wp.tile([1, 16], f32)
            nc.scalar.activation(wact[:], warm[0:1, 0:16], AF.Relu)
            nc.scalar.activation(wact[:], warm[0:1, 0:16], AF.Sigmoid)
            psw = pp.tile([1, NCH], f32, tag="ps", name="psw")
            for _ in range(7):
                nc.tensor.matmul(
                    psw[0:1, 0:512], warm[0:1, 0:1], warm[0:1, :], start=True,
                    stop=True,
                )

            nc.sync.dma_start(w8[:, 8:12, :], w8_d[:, 8:12, :])

            xt_tiles = {}

            def load_xt(b):
                xt = dp.tile([128, 4, 2, NCH], f8, tag="xt", name=f"xt_{b}")
                nc.sync.dma_start(xt[:], xwc_d[b])
                xt_tiles[b] = xt

            load_xt(0)
            nc.sync.dma_start(refer2[:, :, 1024:2048], refer2_d[:, :, 1024:2048])
            nc.sync.dma_start(w8[:, 12:16, :], w8_d[:, 12:16, :])
            load_xt(1)

            res = wp.tile([128, 2048], f16)
            wb = wf32[:, 8:40].bitcast(bf16)  # [128, 64] w3/256

            # psum->sbuf relu ops: only ACT and DVE can read PSUM
            def relu_op(eng, out, psum, bias):
                if eng == "a":
                    nc.scalar.activation(out, psum, AF.Relu, bias=bias)
                else:
                    nc.vector.tensor_scalar(out, psum, bias, 0.0, ALU.add, ALU.max)

            s1r_tiles, s1x_tiles, hcat_tiles = {}, {}, {}

            def conv_ref(s):
                s1r = [
                    s1p.tile([128, 2, NCH], f8, tag="s1", name=f"s1r{g}_{s}")
                    for g in range(2)
                ]
                for q in range(4):
                    ps = pp.tile([128, NCH], f32, tag="ps", name=f"psr_{s}_{q}")
                    for nh in range(4):
                        nc.tensor.matmul(
                            ps[:, 256 * nh:256 * (nh + 1)],
                            w8[:, 2 * q:2 * q + 2, :],
                            refer2[:, :, NCH * s + 256 * nh:NCH * s + 256 * (nh + 1)],
                            start=True, stop=True, perf_mode=DR,
                        )
                    relu_op("av"[q % 2], s1r[q // 2][:, q % 2, :], ps[:],
                            wf32[:, q:q + 1])
                s1r_tiles[s] = s1r

            def conv_x_win(s, j, eng):
                if j == 0:
                    s1x_tiles[s] = [
                        s1p.tile([128, 2, NCH], f8, tag="s1", name=f"s1x{g}_{s}")
                        for g in range(2)
                    ]
                xt = xt_tiles[s]
                ps = pp.tile([128, NCH], f32, tag="ps", name=f"psx_{s}_{j}")
                for nh in range(4):
                    o = 256 * nh
                    nc.tensor.matmul(
                        ps[:, o:o + 256],
                        w8[:, 8:10, :],
                        xt[:, j, :, o:o + 256],
                        start=True, stop=False, perf_mode=DR,
                    )
                    if j < 3:
                        prhs = xt[0:2, j + 1, :, o:o + 256]
                    else:
                        prhs = xrest[0:2, :, NCH * s + o:NCH * s + o + 256]
                    nc.tensor.matmul(
                        ps[:, o:o + 256],
                        w8[0:2, 10:12, :],
                        prhs,
                        start=False, stop=True, perf_mode=DR,
                    )
                relu_op(eng, s1x_tiles[s][j // 2][:, j % 2, :], ps[:],
                        wf32[:, 6:7])

            def mm2_half(ph, s1t, c0, cols):
                """mm2 psum cols [c0, c0+cols) from s1 tiles."""
                for o in range(c0, c0 + cols, 256):
                    for g in range(2):
                        nc.tensor.matmul(
                            ph[:, o:o + 256],
                            w8[:, 12 + 2 * g:14 + 2 * g, :],
                            s1t[g][:, :, o:o + 256],
                            start=(g == 0), stop=(g == 1), perf_mode=DR,
                        )

            def tail_mm2(s):
                hcat = hp.tile([128, 2, NCH], bf16, tag="h", name=f"hcat_{s}")
                hcat_tiles[s] = hcat
                phr = pp.tile([128, NCH], f32, tag="ps", name=f"pshr_{s}")
                mm2_half(phr, s1r_tiles[s], 0, NCH)
                relu_op("a", hcat[:, 0, :], phr[:], wf32[:, 5:6])
                phx = pp.tile([128, NCH], f32, tag="ps", name=f"pshx_{s}")
                mm2_half(phx, s1x_tiles[s], 0, NCH)
                relu_op("v", hcat[:, 1, :], phx[:], wf32[:, 5:6])

            def tail_mm3(s):
                hcat = hcat_tiles[s]
                pso = pp.tile([128, NCH], f32, tag="ps", name=f"pso_{s}")
                for half in range(2):
                    for n2 in range(2):
                        nc.tensor.matmul(
                            pso[64 * half:64 * half + 64, 512 * n2:512 * (n2 + 1)],
                            wb,
                            hcat[:, half, 512 * n2:512 * (n2 + 1)],
                            start=True, stop=True,
                            tile_position=(0, 64 * half),
                        )
                nc.scalar.activation(
                    res[:, NCH * s:NCH * (s + 1)], pso[:], AF.Sigmoid,
                    bias=wf32[:, 4:5],
                )
                nc.sync.dma_start(
                    res_d[:, NCH * s:NCH * (s + 1)], res[:, NCH * s:NCH * (s + 1)]
                )

            def tail_split(s):
                """Last super-pair: half-width hsb ops feed half-width
                mm3/sigmoid/store chains so the program end pipelines."""
                hcat = hp.tile([128, 2, NCH], bf16, tag="h", name=f"hcat_{s}")
                psos = []
                for n2 in range(2):
                    c0 = 512 * n2
                    phr = pp.tile([128, NCH], f32, tag="ps", name=f"pshr_{s}_{n2}")
                    mm2_half(phr, s1r_tiles[s], c0, 512)
                    relu_op("a", hcat[:, 0, c0:c0 + 512], phr[:, c0:c0 + 512],
                            wf32[:, 5:6])
                    phx = pp.tile([128, NCH], f32, tag="ps", name=f"pshx_{s}_{n2}")
                    mm2_half(phx, s1x_tiles[s], c0, 512)
                    relu_op("v", hcat[:, 1, c0:c0 + 512], phx[:, c0:c0 + 512],
                            wf32[:, 5:6])
                for n2 in range(2):
                    c0 = 512 * n2
                    pso = pp.tile([128, NCH], f32, tag="ps", name=f"pso_{s}_{n2}")
                    for half in range(2):
                        nc.tensor.matmul(
                            pso[64 * half:64 * half + 64, c0:c0 + 512],
                            wb,
                            hcat[:, half, c0:c0 + 512],
                            start=True, stop=True,
                            tile_position=(0, 64 * half),
                        )
                    nc.scalar.activation(
                        res[:, NCH * s + c0:NCH * s + c0 + 512],
                        pso[:, c0:c0 + 512], AF.Sigmoid, bias=wf32[:, 4:5],
                    )
                    nc.sync.dma_start(
                        res_d[:, NCH * s + c0:NCH * s + c0 + 512],
                        res[:, NCH * s + c0:NCH * s + c0 + 512],
                    )

            conv_ref(0)
            for j in range(4):
                conv_x_win(0, j, "av"[j % 2])
            conv_ref(1)
            tail_mm2(0)
            conv_x_win(1, 0, "a")
            conv_x_win(1, 1, "v")
            tail_mm3(0)
            conv_x_win(1, 2, "a")
            conv_x_win(1, 3, "v")
            tail_split(1)

    nc.compile()
    return nc


def _host_prep_weights(w_seq, b_seq, w1, b1, w2, b2, w3, b3):
    import ml_dtypes

    E4 = ml_dtypes.float8_e4m3
    BF = ml_dtypes.bfloat16

    w_seq64 = np.asarray(w_seq, np.float64)
    b_seq64 = np.asarray(b_seq, np.float64)
    w164 = np.asarray(w1, np.float64)

    Weff = np.zeros((D_SEQ, D_IN, 4))
    beff = np.full(D_SEQ, float(np.asarray(b1).reshape(-1)[0]))
    for cc in (0, 1):
        for k in range(3):
            dlo, dhi = max(0, 1 - k), min(D_SEQ, D_SEQ + 1 - k)
            for kk in range(3):
                tau = cc + kk
                Weff[dlo:dhi, :, tau] += (
                    w164[0, cc, k] * w_seq64[dlo + k - 1:dhi + k - 1, :, kk]
                )
    for k in range(3):
        dlo, dhi = max(0, 1 - k), min(D_SEQ, D_SEQ + 1 - k)
        beff[dlo:dhi] += (w164[0, 0, k] + w164[0, 1, k]) * b_seq64[dlo + k - 1:dhi + k - 1]

    w8 = np.zeros((128, 16, 128), np.float64)
    # wefft groups 2q+i: w8[64*bb+ch, 2q+i, m] = 16*Weff[128q+m, ch, 2bb+i]
    for q in range(4):
        for bb in range(2):
            for i in range(2):
                w8[64 * bb:64 * bb + 64, 2 * q + i, :] = (
                    16.0 * Weff[128 * q:128 * (q + 1), :, 2 * bb + i].T
                )
    # band groups 8+i: w8[p, 8+i, m] = 16*w1[0,i,p-m], p-m in {0,1,2}
    for i in range(2):
        for m in range(128):
            for k in range(3):
                if m + k < 128:
                    w8[m + k, 8 + i, m] = 16.0 * w164[0, i, k]
        # patch groups 10+i
        w8[0, 10 + i, 126] = 16.0 * w164[0, i, 2]
        w8[0, 10 + i, 127] = 16.0 * w164[0, i, 1]
        w8[1, 10 + i, 127] = 16.0 * w164[0, i, 2]
    # w2 groups 12+2g+i: w8[p, 12+2g+i, e] = 16*w2[e, 128*(2g+i)+p]
    w2m = np.asarray(w2, np.float64)[:, :, 1]  # (128, 512)
    for g in range(2):
        for i in range(2):
            blk = 2 * g + i
            w8[:, 12 + 2 * g + i, :] = 16.0 * w2m[:, 128 * blk:128 * (blk + 1)].T

    wf32 = np.zeros((128, 40), np.float32)
    for q in range(4):
        wf32[:, q] = 16.0 * beff[128 * q:128 * (q + 1)]
    b3a = np.asarray(b3, np.float64)
    wf32[:, 4] = np.tile(b3a, 2).astype(np.float32)
    wf32[:, 5] = 256.0 * np.asarray(b2, np.float64)
    wf32[:, 6] = 16.0 * float(np.asarray(b1).reshape(-1)[0])
    # w3' = w3/256 as bf16, bit-packed into f32 columns 8..40
    w3p = np.ascontiguousarray((np.asarray(w3, np.float64)[:, :, 1].T / 256.0)).astype(BF)
    wf32[:, 8:40] = w3p.view(np.float32)

    return np.ascontiguousarray(w8.astype(E4)), np.ascontiguousarray(wf32)


def _host_prep_data(refer, x):
    """Global fp8 conversion + padded transposes shared by all cores."""
    import ml_dtypes

    E4 = ml_dtypes.float8_e4m3
    refer8p = np.zeros((D_IN, T_REF + 4), E4)
    refer8p[:, 1:T_REF + 1] = np.asarray(refer[0], np.float32).astype(E4)
    x8 = np.asarray(x[0], np.float32).astype(E4)  # (T, 512)
    xTpad = np.zeros((513, T_REF), E4)
    xTpad[1:513] = x8.T
    return refer8p, xTpad


def _host_prep_core(c, refer8p, xTpad):
    import ml_dtypes

    E4 = ml_dtypes.float8_e4m3
    # refer2[p, i, cn] = refer[ch, 4096c + 2cn + i - 1 + 2*(p>=64)]
    # refer8p col t -> index t+1
    refer2 = np.zeros((128, 2, 2048), E4)
    base = 4096 * c
    for i in range(2):
        refer2[0:64, i, :] = refer8p[:, base + i:base + i + 4096:2]
        refer2[64:128, i, :] = refer8p[:, base + i + 2:base + i + 4096 + 2:2]
    # xwc[b, p, j, i, n] = xTpad[128j + p, 4096c + 2048b + 2n + i]
    xwc = np.zeros((2, 128, 4, 2, NCH), E4)
    for b in range(2):
        t0 = 4096 * c + 2048 * b
        for j in range(4):
            blk = xTpad[128 * j:128 * j + 128, t0:t0 + 2048]
            xwc[b, :, j, 0, :] = blk[:, 0::2]
            xwc[b, :, j, 1, :] = blk[:, 1::2]
    xrest = np.zeros((2, 2, 2048), E4)
    xrest[0, 0, :] = xTpad[512, 4096 * c:4096 * (c + 1):2]
    xrest[0, 1, :] = xTpad[512, 4096 * c + 1:4096 * (c + 1):2]
    return refer2, np.ascontiguousarray(xwc), xrest


def kernel(refer, x, w_seq, b_seq, w1, b1, w2, b2, w3, b3):
    from concourse.bass_utils import run_bass_kernel_spmd

    if "nc" not in _CACHE:
        _CACHE["nc"] = _build_nc()
    nc = _CACHE["nc"]

    w8, wf32 = _host_prep_weights(w_seq, b_seq, w1, b1, w2, b2, w3, b3)
    refer8p, xTpad = _host_prep_data(refer, x)
    in_maps = []
    for c in range(N_CORES):
        refer2, xwc, xrest = _host_prep_core(c, refer8p, xTpad)
        in_maps.append(
            dict(refer2=refer2, xwc=xwc, xrest=xrest, w8=w8, wf32=wf32)
        )

    res = run_bass_kernel_spmd(nc, in_maps, core_ids=list(range(N_CORES)))

    final = np.zeros((32768, D_OUT, 1), np.float32)
    for c in range(N_CORES):
        r = np.asarray(res.results[c]["res"], np.float32)  # (128, 2048)
        final[2048 * c:2048 * (c + 1), :, 0] = r[0:64, :].T
        final[16384 + 2048 * c:16384 + 2048 * (c + 1), :, 0] = r[64:128, :].T
    return final


# revision 12
# speedup vs baseline: 1.2087x; 1.0561x over previous
"""Trainium2 Bass kernel for nn_DSNet (dense_cnn) — fp8 DoubleRow version.

Math (see reference): out = sigmoid(relu(relu(conv1(seq_splits)) @ W2 + b2) @ W3 + b3)
where seq = [conv1d(refer, w_seq) ; x^T] split into 32768 (2,512) splits.

Mapping (per core, 4096 splits = 2 super-pairs of (1024 ref + 1024 x) splits):
- ref half: linear_seq+concat+conv1 fold into one stride-2 4-tap conv with
  host-precomputed Weff[d, ch, tau]; computed as DoubleRow fp8 matmuls that
  contract (ch x 2 shifts) x (2 taus in the k-tile dim) = 256 per pass.
- x half: conv1 along the 512-dim becomes 4 banded DoubleRow matmuls with
  M=128 windows + tiny edge-patch matmuls; the 2 k-tiles carry the even/odd
  time rows (host deinterleaves x columns).
- mm2 contracts 512 via DoubleRow over d-block pairs (s1 stored fp8 x16 with
  block pairs side by side in the free dim); mm3 runs bf16 with w3/256 packed
  bit-wise into the f32 bias bundle; sigmoid output fp16.

Scales (powers of 2, exact): refer/x x1 (e4m3), Weff/w1/w2 x16, s1 fp8 x16,
h bf16 x256, w3' = w3/256, biases pre-scaled on host. Measured max rel err
~6.8e-3 vs the f32 reference.

All heavy matmuls are fp8e4 DoubleRow (0.5 cyc/row): ~2.1us ref conv +
~3.4us x conv + ~1.7us mm2 + ~1.7us mm3 of PE per core. Elementwise
(relu/hsb/sigmoid) is spread across Pool/DVE/ACT. DMA ~3.4MB/core fp8.
"""
import sys

import numpy as np

sys.path.insert(0, "/opt/trn_rl_repo")

D_IN, D_SEQ, D_H, D_OUT = 64, 512, 128, 64
T_REF = 32768
N_CORES = 8
NCH = 1024  # splits per chunk (super-pair = ref chunk + x chunk)

_CACHE = {}


def _build_nc():
    import concourse.bacc as bacc
    import concourse.bass as bass
    import concourse.mybir as mybir
    import concourse.tile as tile

    f32 = mybir.dt.float32
    bf16 = mybir.dt.bfloat16
    f16 = mybir.dt.float16
    f8 = mybir.dt.float8e4
    AF = mybir.ActivationFunctionType
    ALU = mybir.AluOpType
    DR = mybir.MatmulPerfMode.DoubleRow

    nc = bacc.Bacc("TRN2", target_bir_lowering=False, debug=False, num_devices=N_CORES)

    refer2_d = nc.dram_tensor("refer2", [128, 2, 2048], f8, kind="ExternalInput").ap()
    xwc_d = nc.dram_tensor("xwc", [2, 128, 4, 2, NCH], f8, kind="ExternalInput").ap()
    xrest_d = nc.dram_tensor("xrest", [2, 2, 2048], f8, kind="ExternalInput").ap()
    w8_d = nc.dram_tensor("w8", [128, 16, 128], f8, kind="ExternalInput").ap()
    wf32_d = nc.dram_tensor("wf32", [128, 40], f32, kind="ExternalInput").ap()
    res_d = nc.dram_tensor("res", [128, 2048], f16, kind="ExternalOutput").ap()

    with tile.TileContext(nc) as tc:
        with (
            tc.tile_pool(name="wp", bufs=1) as wp,
            tc.tile_pool(name="dp", bufs=2) as dp,
            tc.tile_pool(name="s1p", bufs=8) as s1p,
            tc.tile_pool(name="hp", bufs=2) as hp,
            tc.tile_pool(name="pp", bufs=4, space=bass.MemorySpace.PSUM) as pp,
        ):
            # --- PE / ACT warmup: dummy matmuls ramp the PE pstate and the
            # activations preload the Relu/Sigmoid tables while DMAs fly.
            warm = wp.tile([1, 512], f8)
            nc.gpsimd.memset(warm[:], 0.0)
            xrest = wp.tile([2, 2, 2048], f8)
            nc.gpsimd.dma_start(xrest[:], xrest_d[:])

            # critical-path loads on SP/HWDGE in first-needed order
            w8 = wp.tile([128, 16, 128], f8)
            refer2 = wp.tile([128, 2, 2048], f8)
            wf32 = wp.tile([128, 40], f32)
            nc.sync.dma_start(w8[:, 0:12, :], w8_d[:, 0:12, :])
            nc.sync.dma_start(refer2[:, :, 0:1024], refer2_d[:, :, 0:1024])
            nc.sync.dma_start(wf32[:], wf32_d[:])

            wact = wp.tile([1, 16], f32)
            nc.scalar.activation(wact[:], warm[0:1, 0:16], AF.Relu)
            nc.scalar.activation(wact[:], warm[0:1, 0:16], AF.Sigmoid)
            psw = pp.tile([1, NCH], f32, tag="ps", name="psw")
            for _ in range(7):
                nc.tensor.matmul(
                    psw[0:1, 0:512], warm[0:1, 0:1], warm[0:1, :], start=True,
                    stop=True,
                )

            xt_tiles = {}

            def load_xt(b):
                xt = dp.tile([128, 4, 2, NCH], f8, tag="xt", name=f"xt_{b}")
                nc.sync.dma_start(xt[:, 0:2], xwc_d[b, :, 0:2])
                nc.sync.dma_start(xt[:, 2:4], xwc_d[b, :, 2:4])
                xt_tiles[b] = xt

            load_xt(0)
            nc.sync.dma_start(refer2[:, :, 1024:2048], refer2_d[:, :, 1024:2048])
            nc.sync.dma_start(w8[:, 12:16, :], w8_d[:, 12:16, :])
            load_xt(1)

            res = wp.tile([128, 2048], f16)
            wb = wf32[:, 8:40].bitcast(bf16)  # [128, 64] w3/256

            # psum->sbuf relu ops: only ACT and DVE can read PSUM
            def relu_op(eng, out, psum, bias):
                if eng == "a":
                    nc.scalar.activation(out, psum, AF.Relu, bias=bias)
                else:
                    nc.vector.tensor_scalar(out, psum, bias, 0.0, ALU.add, ALU.max)

            s1r_tiles, s1x_tiles, hcat_tiles = {}, {}, {}

            def conv_ref(s):
                s1r = [
                    s1p.tile([128, 2, NCH], f8, tag="s1", name=f"s1r{g}_{s}")
                    for g in range(2)
                ]
                for q in range(4):
                    ps = pp.tile([128, NCH], f32, tag="ps", name=f"psr_{s}_{q}")
                    for nh in range(4):
                        nc.tensor.matmul(
                            ps[:, 256 * nh:256 * (nh + 1)],
                            w8[:, 2 * q:2 * q + 2, :],
                            refer2[:, :, NCH * s + 256 * nh:NCH * s + 256 * (nh + 1)],
                            start=True, stop=True, perf_mode=DR,
                        )
                    relu_op("av"[q % 2], s1r[q // 2][:, q % 2, :], ps[:],
                            wf32[:, q:q + 1])
                s1r_tiles[s] = s1r

            def conv_x_win(s, j, eng):
                if j == 0:
                    s1x_tiles[s] = [
                        s1p.tile([128, 2, NCH], f8, tag="s1", name=f"s1x{g}_{s}")
                        for g in range(2)
                    ]
                xt = xt_tiles[s]
                ps = pp.tile([128, NCH], f32, tag="ps", name=f"psx_{s}_{j}")
                for nh in range(4):
                    o = 256 * nh
                    nc.tensor.matmul(
                        ps[:, o:o + 256],
                        w8[:, 8:10, :],
                        xt[:, j, :, o:o + 256],
                        start=True, stop=False, perf_mode=DR,
                    )
                    if j < 3:
                        prhs = xt[0:2, j + 1, :, o:o + 256]
                    else:
                        prhs = xrest[0:2, :, NCH * s + o:NCH * s + o + 256]
                    nc.tensor.matmul(
                        ps[:, o:o + 256],
                        w8[0:2, 10:12, :],
                        prhs,
                        start=False, stop=True, perf_mode=DR,
                    )
                relu_op(eng, s1x_tiles[s][j // 2][:, j % 2, :], ps[:],
                        wf32[:, 6:7])

            def mm2_half(ph, s1t, c0, cols):
                """mm2 psum cols [c0, c0+cols) from s1 tiles."""
                for o in range(c0, c0 + cols, 256):
                    for g in range(2):
                        nc.tensor.matmul(
                            ph[:, o:o + 256],
                            w8[:, 12 + 2 * g:14 + 2 * g, :],
                            s1t[g][:, :, o:o + 256],
                            start=(g == 0), stop=(g == 1), perf_mode=DR,
                        )

            def tail_mm2(s):
                hcat = hp.tile([128, 2, NCH], bf16, tag="h", name=f"hcat_{s}")
                hcat_tiles[s] = hcat
                phr = pp.tile([128, NCH], f32, tag="ps", name=f"pshr_{s}")
                mm2_half(phr, s1r_tiles[s], 0, NCH)
                relu_op("a", hcat[:, 0, :], phr[:], wf32[:, 5:6])
                phx = pp.tile([128, NCH], f32, tag="ps", name=f"pshx_{s}")
                mm2_half(phx, s1x_tiles[s], 0, NCH)
                relu_op("v", hcat[:, 1, :], phx[:], wf32[:, 5:6])

            def tail_mm3(s):
                hcat = hcat_tiles[s]
                pso = pp.tile([128, NCH], f32, tag="ps", name=f"pso_{s}")
                for half in range(2):
                    for n2 in range(2):
                        nc.tensor.matmul(
                            pso[64 * half:64 * half + 64, 512 * n2:512 * (n2 + 1)],
                            wb,
                            hcat[:, half, 512 * n2:512 * (n2 + 1)],
                            start=True, stop=True,
                            tile_position=(0, 64 * half),
                        )
                nc.scalar.activation(
                    res[:, NCH * s:NCH * (s + 1)], pso[:], AF.Sigmoid,
                    bias=wf32[:, 4:5],
                )
                nc.sync.dma_start(
                    res_d[:, NCH * s:NCH * (s + 1)], res[:, NCH * s:NCH * (s + 1)]
                )

            def tail_split(s):
                """Last super-pair: half-width hsb ops feed half-width
                mm3/sigmoid/store chains so the program end pipelines."""
                hcat = hp.tile([128, 2, NCH], bf16, tag="h", name=f"hcat_{s}")
                psos = []
                for n2 in range(2):
                    c0 = 512 * n2
                    phr = pp.tile([128, NCH], f32, tag="ps", name=f"pshr_{s}_{n2}")
                    mm2_half(phr, s1r_tiles[s], c0, 512)
                    relu_op("a", hcat[:, 0, c0:c0 + 512], phr[:, c0:c0 + 512],
                            wf32[:, 5:6])
                    phx = pp.tile([128, NCH], f32, tag="ps", name=f"pshx_{s}_{n2}")
                    mm2_half(phx, s1x_tiles[s], c0, 512)
                    relu_op("v", hcat[:, 1, c0:c0 + 512], phx[:, c0:c0 + 512],
                            wf32[:, 5:6])
                for n2 in range(2):
                    c0 = 512 * n2
                    pso = pp.tile([128, NCH], f32, tag="ps", name=f"pso_{s}_{n2}")
                    for half in range(2):
                        nc.tensor.matmul(
                            pso[64 * half:64 * half + 64, c0:c0 + 512],
                            wb,
                            hcat[:, half, c0:c0 + 512],
                            start=True, stop=True,
                            tile_position=(0, 64 * half),
                        )
                    nc.scalar.activation(
                        res[:, NCH * s + c0:NCH * s + c0 + 512],
                        pso[:, c0:c0 + 512], AF.Sigmoid, bias=wf32[:, 4:5],
                    )
                    nc.sync.dma_start(
                        res_d[:, NCH * s + c0:NCH * s + c0 + 512],
                        res[:, NCH * s + c0:NCH * s + c0 + 512],
                    )

            conv_ref(0)
            for j in range(4):
                conv_x_win(0, j, "av"[j % 2])
            conv_ref(1)
            tail_mm2(0)
            conv_x_win(1, 0, "a")
            conv_x_win(1, 1, "v")
            conv_x_win(1, 2, "a")
            conv_x_win(1, 3, "v")
            tail_mm3(0)
            tail_split(1)

    nc.compile()
    return nc


def _host_prep_weights(w_seq, b_seq, w1, b1, w2, b2, w3, b3):
    import ml_dtypes

    E4 = ml_dtypes.float8_e4m3
    BF = ml_dtypes.bfloat16

    w_seq64 = np.asarray(w_seq, np.float64)
    b_seq64 = np.asarray(b_seq, np.float64)
    w164 = np.asarray(w1, np.float64)

    Weff = np.zeros((D_SEQ, D_IN, 4))
    beff = np.full(D_SEQ, float(np.asarray(b1).reshape(-1)[0]))
    for cc in (0, 1):
        for k in range(3):
            dlo, dhi = max(0, 1 - k), min(D_SEQ, D_SEQ + 1 - k)
            for kk in range(3):
                tau = cc + kk
                Weff[dlo:dhi, :, tau] += (
                    w164[0, cc, k] * w_seq64[dlo + k - 1:dhi + k - 1, :, kk]
                )
    for k in range(3):
        dlo, dhi = max(0, 1 - k), min(D_SEQ, D_SEQ + 1 - k)
        beff[dlo:dhi] += (w164[0, 0, k] + w164[0, 1, k]) * b_seq64[dlo + k - 1:dhi + k - 1]

    w8 = np.zeros((128, 16, 128), np.float64)
    # wefft groups 2q+i: w8[64*bb+ch, 2q+i, m] = 16*Weff[128q+m, ch, 2bb+i]
    for q in range(4):
        for bb in range(2):
            for i in range(2):
                w8[64 * bb:64 * bb + 64, 2 * q + i, :] = (
                    16.0 * Weff[128 * q:128 * (q + 1), :, 2 * bb + i].T
                )
    # band groups 8+i: w8[p, 8+i, m] = 16*w1[0,i,p-m], p-m in {0,1,2}
    for i in range(2):
        for m in range(128):
            for k in range(3):
                if m + k < 128:
                    w8[m + k, 8 + i, m] = 16.0 * w164[0, i, k]
        # patch groups 10+i
        w8[0, 10 + i, 126] = 16.0 * w164[0, i, 2]
        w8[0, 10 + i, 127] = 16.0 * w164[0, i, 1]
        w8[1, 10 + i, 127] = 16.0 * w164[0, i, 2]
    # w2 groups 12+2g+i: w8[p, 12+2g+i, e] = 16*w2[e, 128*(2g+i)+p]
    w2m = np.asarray(w2, np.float64)[:, :, 1]  # (128, 512)
    for g in range(2):
        for i in range(2):
            blk = 2 * g + i
            w8[:, 12 + 2 * g + i, :] = 16.0 * w2m[:, 128 * blk:128 * (blk + 1)].T

    wf32 = np.zeros((128, 40), np.float32)
    for q in range(4):
        wf32[:, q] = 16.0 * beff[128 * q:128 * (q + 1)]
    b3a = np.asarray(b3, np.float64)
    wf32[:, 4] = np.tile(b3a, 2).astype(np.float32)
    wf32[:, 5] = 256.0 * np.asarray(b2, np.float64)
    wf32[:, 6] = 16.0 * float(np.asarray(b1).reshape(-1)[0])
    # w3' = w3/256 as bf16, bit-packed into f32 columns 8..40
    w3p = np.ascontiguousarray((np.asarray(w3, np.float64)[:, :, 1].T / 256.0)).astype(BF)
    wf32[:, 8:40] = w3p.view(np.float32)

    return np.ascontiguousarray(w8.astype(E4)), np.ascontiguousarray(wf32)


def _host_prep_data(refer, x):
    """Global fp8 conversion + padded transposes shared by all cores."""
    import ml_dtypes

    E4 = ml_dtypes.float8_e4m3
    refer8p = np.zeros((D_IN, T_REF + 4), E4)
    refer8p[:, 1:T_REF + 1] = np.asarray(refer[0], np.float32).astype(E4)
    x8 = np.asarray(x[0], np.float32).astype(E4)  # (T, 512)
    xTpad = np.zeros((513, T_REF), E4)
    xTpad[1:513] = x8.T
    return refer8p, xTpad


def _host_prep_core(c, refer8p, xTpad):
    import ml_dtypes

    E4 = ml_dtypes.float8_e4m3
    # refer2[p, i, cn] = refer[ch, 4096c + 2cn + i - 1 + 2*(p>=64)]
    # refer8p col t -> index t+1
    refer2 = np.zeros((128, 2, 2048), E4)
    base = 4096 * c
    for i in range(2):
        refer2[0:64, i, :] = refer8p[:, base + i:base + i + 4096:2]
        refer2[64:128, i, :] = refer8p[:, base + i + 2:base + i + 4096 + 2:2]
    # xwc[b, p, j, i, n] = xTpad[128j + p, 4096c + 2048b + 2n + i]
    xwc = np.zeros((2, 128, 4, 2, NCH), E4)
    for b in range(2):
        t0 = 4096 * c + 2048 * b
        for j in range(4):
            blk = xTpad[128 * j:128 * j + 128, t0:t0 + 2048]
            xwc[b, :, j, 0, :] = blk[:, 0::2]
            xwc[b, :, j, 1, :] = blk[:, 1::2]
    xrest = np.zeros((2, 2, 2048), E4)
    xrest[0, 0, :] = xTpad[512, 4096 * c:4096 * (c + 1):2]
    xrest[0, 1, :] = xTpad[512, 4096 * c + 1:4096 * (c + 1):2]
    return refer2, np.ascontiguousarray(xwc), xrest


def kernel(refer, x, w_seq, b_seq, w1, b1, w2, b2, w3, b3):
    from concourse.bass_utils import run_bass_kernel_spmd

    if "nc" not in _CACHE:
        _CACHE["nc"] = _build_nc()
    nc = _CACHE["nc"]

    w8, wf32 = _host_prep_weights(w_seq, b_seq, w1, b1, w2, b2, w3, b3)
    refer8p, xTpad = _host_prep_data(refer, x)
    in_maps = []
    for c in range(N_CORES):
        refer2, xwc, xrest = _host_prep_core(c, refer8p, xTpad)
        in_maps.append(
            dict(refer2=refer2, xwc=xwc, xrest=xrest, w8=w8, wf32=wf32)
        )

    res = run_bass_kernel_spmd(nc, in_maps, core_ids=list(range(N_CORES)))

    final = np.zeros((32768, D_OUT, 1), np.float32)
    for c in range(N_CORES):
        r = np.asarray(res.results[c]["res"], np.float32)  # (128, 2048)
        final[2048 * c:2048 * (c + 1), :, 0] = r[0:64, :].T
        final[16384 + 2048 * c:16384 + 2048 * (c + 1), :, 0] = r[64:128, :].T
    return final
